# revision 1
# baseline (speedup 1.0000x reference)
"""Causal self-attention (B=2, T=2048, C=768, H=12) on 8 NeuronCores.

Sharding: zero-collective B x query-chunk sharding. Core d handles batch
b = d//4 and the causally-balanced query-chunk pair j = d%4:
rows [256j, 256j+256) and [256(7-j), 256(7-j)+256). Each core computes
K/V for the full sequence locally (redundant but collective-free), all 12
heads for its 512 query rows, and the output projection for its rows.
Output gather on host is a pure concat.

On-device pipeline (bf16 matmul operands, fp32 PSUM accumulation):
  xT = PE-transpose(x)            ->  Q^T/K^T via W-stationary matmuls
  S^T[k,q] = K^T(h)^T Q^T(h)      ->  exp on ACT (scale=1/8, no max-sub;
  |logits| <= ~20 so f32 exp is safe) -> causal mask multiply (host-built
  mask tiles) -> P^T @ [V|1] folds the softmax denominator into row 64 of
  the PSUM accumulator -> reciprocal + K=1 ones-matmul broadcast ->
  normalize into Y^T -> output projection + bias.

Engine discipline: ACT runs only Exp (activation-table swaps are
expensive); every PSUM->SBUF copy-out runs on DVE. Steps 0-7 process
both query chunks in one fused [128,512] tile; steps 8-15 are the
chunk-B tail.
"""

import numpy as np
import ml_dtypes

B, T, C, H, D = 2, 2048, 768, 12, 64
NCORES = 8
QCH = 256          # query chunk width
NSTEPS_A, NSTEPS_B = 8, 16   # uniform k-tile extents for chunk A / B

_CACHE = {}


def _build_program(with_bias=True):
    import os
    KOPT = set(os.environ.get("KOPT", "").split(","))
    import concourse.bass as bass
    import concourse.bacc as bacc
    import concourse.mybir as mybir
    import concourse.tile as tile

    F32 = mybir.dt.float32
    BF16 = mybir.dt.bfloat16
    AF = mybir.ActivationFunctionType

    nc = bacc.Bacc()
    xbf = nc.declare_dram_parameter("xbf", [T, C], BF16, isOutput=False)
    xqbf = nc.declare_dram_parameter("xqbf", [2 * QCH, C], BF16, isOutput=False)
    wqkv = nc.declare_dram_parameter("wqkv", [C, 3 * C], BF16, isOutput=False)
    wproj = nc.declare_dram_parameter("wproj", [C, C], BF16, isOutput=False)
    bqkv = nc.declare_dram_parameter("bqkv", [128, 18], F32, isOutput=False)
    bproj = nc.declare_dram_parameter("bproj", [1, C], F32, isOutput=False)
    masks = nc.declare_dram_parameter("masks", [128, NSTEPS_B, 2 * QCH], BF16,
                                      isOutput=False)
    ident_in = nc.declare_dram_parameter("ident_in", [128, 128], BF16,
                                         isOutput=False)
    z_out = nc.declare_dram_parameter("z", [2 * QCH, C], F32, isOutput=True)

    CT = C // 128            # 6 c-tiles
    TT = T // 128            # 16 t-tiles
    QT_N = 2 * QCH           # 512 own query rows

    with tile.TileContext(nc) as tc:
        with tc.tile_pool(name="const", bufs=1) as constp, \
             tc.tile_pool(name="data", bufs=1) as datap, \
             tc.tile_pool(name="work", bufs=2) as workp, \
             tc.tile_pool(name="pt", bufs=6) as ptp, \
             tc.tile_pool(name="small", bufs=2) as smallp, \
             tc.tile_pool(name="zs", bufs=2) as zsp, \
             tc.tile_pool(name="ps", bufs=int(os.environ.get("KPS", "2")),
                          space="PSUM") as psp, \
             tc.tile_pool(name="pso", bufs=2, space="PSUM") as psop:

            # ---- constants (declared; DMAs emitted in priority order) -----
            masks_s = constp.tile([128, NSTEPS_B, 2 * QCH], BF16, tag="masks")
            bqkv_s = constp.tile([128, 18], F32, tag="bqkv")
            bpb = constp.tile([128, C], F32, tag="bpb")
            ones1 = constp.tile([1, 64], BF16, tag="ones1")
            nc.vector.memset(ones1, 1.0)
            wq_s = constp.tile([128, CT, 3 * C], BF16, tag="wqkv")
            wp_s = constp.tile([128, CT, C], BF16, tag="wproj")
            idn = constp.tile([128, 128], BF16, tag="idn")
            w_ap = wqkv[:, :]
            wp_ap = wproj[:, :]
            bp_ap = bproj[:, :]

            # ---- persistent tiles -----------------------------------------
            xT = [datap.tile([128, T], BF16, tag=f"xT{c}", name=f"xT{c}")
                  for c in range(CT)]
            xqT = [datap.tile([128, QT_N], BF16, tag=f"xqT{c}", name=f"xqT{c}")
                   for c in range(CT)]
            KT = [datap.tile([128, T], BF16, tag=f"KT{m}", name=f"KT{m}")
                  for m in range(CT)]
            QTt = [datap.tile([128, QT_N], BF16, tag=f"QT{m}", name=f"QT{m}")
                   for m in range(CT)]
            V = [datap.tile([128, H, D + 1], BF16, tag=f"V{t}", name=f"V{t}")
                 for t in range(TT)]
            YT = [datap.tile([128, QT_N], BF16, tag=f"YT{m}", name=f"YT{m}")
                  for m in range(CT)]

            # ---- phase A: hybrid transposes, priority-ordered DMAs --------
            # sync queue: idn -> x c-block quarters 0-2 -> W(K) by m-col
            # scalar queue: xbar transposes xT 3-5, xq 0-5, then W(V), W(Q)
            # gpsimd SWDGE queue: bqkv, bproj, Wproj, masks (non-critical)
            nc.sync.dma_start(out=idn, in_=ident_in[:, :])
            for c in range(3, CT):
                nc.scalar.dma_start_transpose(
                    out=xT[c], in_=xbf[:, 128 * c:128 * (c + 1)])
            for c in range(CT):
                nc.scalar.dma_start_transpose(
                    out=xqT[c], in_=xqbf[:, 128 * c:128 * (c + 1)])
            xls = []
            for c in range(3):
                xl = workp.tile([128, TT, 128], BF16, tag=f"xld{c}",
                                name=f"xld{c}")
                nc.sync.dma_start(
                    out=xl,
                    in_=bass.AP(tensor=xbf[:, :].tensor,
                                offset=xbf[:, :].offset + 128 * c,
                                ap=[[C, 128], [128 * C, TT], [1, 128]]),
                )
                xls.append(xl)
            for (lo, hi) in ((2 * C, 2 * C + 512), (2 * C + 512, 3 * C)):
                nc.sync.dma_start(
                    out=wq_s[:, :, lo:hi],
                    in_=bass.AP(tensor=w_ap.tensor, offset=w_ap.offset + lo,
                                ap=[[3 * C, 128], [128 * 3 * C, CT],
                                    [1, hi - lo]]),
                )
            for m in range(CT):
                nc.gpsimd.dma_start(
                    out=wq_s[:, :, C + 128 * m:C + 128 * (m + 1)],
                    in_=bass.AP(tensor=w_ap.tensor,
                                offset=w_ap.offset + C + 128 * m,
                                ap=[[3 * C, 128], [128 * 3 * C, CT],
                                    [1, 128]]),
                )
            for m in range(CT):
                nc.gpsimd.dma_start(
                    out=wq_s[:, :, 128 * m:128 * (m + 1)],
                    in_=bass.AP(tensor=w_ap.tensor,
                                offset=w_ap.offset + 128 * m,
                                ap=[[3 * C, 128], [128 * 3 * C, CT],
                                    [1, 128]]),
                )
            if with_bias:
                nc.gpsimd.dma_start(out=bqkv_s, in_=bqkv[:, :])
                nc.gpsimd.dma_start(
                    out=bpb,
                    in_=bass.AP(tensor=bp_ap.tensor, offset=bp_ap.offset,
                                ap=[[0, 128], bp_ap.ap[1]]),
                )
            nc.gpsimd.dma_start(
                out=wp_s,
                in_=bass.AP(tensor=wp_ap.tensor, offset=wp_ap.offset,
                            ap=[[C, 128], [128 * C, CT], wp_ap.ap[1]]),
            )
            nc.gpsimd.dma_start(out=masks_s, in_=masks[:, :, :])
            for c in range(3):
                for t in range(TT):
                    tp = psp.tile([128, 128], BF16, tag="s", name="tp")
                    nc.tensor.transpose(out=tp, in_=xls[c][:, t, :],
                                        identity=idn)
                    nc.vector.tensor_copy(
                        out=xT[c][:, 128 * t:128 * (t + 1)], in_=tp)

            # ---- phases B+C interleaved -----------------------------------
            # V first (needed by every pair); then per pair m: KT(m), QT(m),
            # the pair's attention steps, and the deferred normalize of the
            # previous pair. Keeps PE fed during the ACT-heavy attention.
            for t in range(TT):
                for (off, w) in ((0, 512), (512, 256)):
                    acc = psp.tile([128, 512], F32, tag="acc", name="acc")
                    corder = (3, 4, 5, 0, 1, 2)
                    for ci, c in enumerate(corder):
                        nc.tensor.matmul(
                            out=acc[:, 0:w],
                            lhsT=xT[c][:, 128 * t:128 * (t + 1)],
                            rhs=wq_s[:, c, 2 * C + off:2 * C + off + w],
                            start=(ci == 0), stop=(ci == CT - 1))
                    h0 = off // D
                    nc.vector.tensor_copy(
                        out=V[t][:, h0:h0 + w // D, 0:D], in_=acc[:, 0:w])
                nc.vector.memset(V[t][:, :, D:D + 1], 1.0)

            scale = 1.0 / float(np.sqrt(D))

            def normalize_pair(m, ots):
                for i in range(2):
                    po = 64 * i
                    ot = ots[i]
                    for qc in range(2):
                        csl = slice(QCH * qc, QCH * (qc + 1))
                        rec = smallp.tile([1, QCH], F32, tag="rec",
                                          name="rec")
                        nc.vector.reciprocal(out=rec, in_=ot[64:65, csl])
                        recbf = smallp.tile([1, QCH], BF16, tag="recbf",
                                            name="recbf")
                        nc.vector.tensor_copy(out=recbf, in_=rec)
                        recb = psp.tile([64, QCH], F32, tag="acc",
                                        name="recb")
                        nc.tensor.matmul(out=recb, lhsT=ones1, rhs=recbf,
                                         start=True, stop=True)
                        recb_sb = smallp.tile([64, QCH], F32,
                                              tag="recb_sb", name="recb_sb")
                        nc.vector.tensor_copy(out=recb_sb, in_=recb)
                        ysl = YT[m][po:po + 64, csl]
                        nc.vector.tensor_mul(ysl, ot[0:64, csl], recb_sb)
                        if with_bias:
                            nc.vector.tensor_scalar_add(
                                ysl, in0=ysl,
                                scalar1=bqkv_s[po:po + 64, 12 + m:13 + m])

            pending = None
            for m in range(CT):
                for n in range(T // 512):
                    acc = psp.tile([128, 512], F32, tag="acc", name="acc")
                    for c in range(CT):
                        nc.tensor.matmul(
                            out=acc,
                            lhsT=wq_s[:, c, C + 128 * m:C + 128 * (m + 1)],
                            rhs=xT[c][:, 512 * n:512 * (n + 1)],
                            start=(c == 0), stop=(c == CT - 1))
                    if with_bias:
                        nc.vector.tensor_scalar_add(
                            KT[m][:, 512 * n:512 * (n + 1)], in0=acc,
                            scalar1=bqkv_s[:, 6 + m:7 + m])
                    else:
                        nc.vector.tensor_copy(
                            out=KT[m][:, 512 * n:512 * (n + 1)], in_=acc)
                acc = psp.tile([128, 512], F32, tag="acc", name="acc")
                for c in range(CT):
                    nc.tensor.matmul(
                        out=acc,
                        lhsT=wq_s[:, c, 128 * m:128 * (m + 1)],
                        rhs=xqT[c],
                        start=(c == 0), stop=(c == CT - 1))
                if with_bias:
                    nc.vector.tensor_scalar_add(
                        QTt[m], in0=acc, scalar1=bqkv_s[:, m:m + 1])
                else:
                    nc.vector.tensor_copy(out=QTt[m], in_=acc)

                ots = [psop.tile([65, 2 * QCH], F32, tag=f"ot{i}",
                                 name=f"ot{i}") for i in range(2)]
                for s in range(NSTEPS_B):
                    w = 512 if s < NSTEPS_A else 256
                    qsl = (slice(0, 512) if s < NSTEPS_A
                           else slice(QCH, 2 * QCH))
                    pts = []
                    for i, po in ((0, 0), (1, 64)):
                        sps = psp.tile([128, 512], F32, tag="s", name="sps")
                        nc.tensor.matmul(
                            out=sps[:, 0:w],
                            lhsT=KT[m][po:po + 64, 128 * s:128 * (s + 1)],
                            rhs=QTt[m][po:po + 64, qsl],
                            start=True, stop=True)
                        pt = ptp.tile([128, 512], BF16, tag="pt", name="pt")
                        nc.scalar.activation(out=pt[:, 0:w], in_=sps[:, 0:w],
                                             func=AF.Exp, scale=scale)
                        nc.vector.tensor_mul(pt[:, 0:QCH], pt[:, 0:QCH],
                                             masks_s[:, s, 0:QCH])
                        pts.append(pt)
                    for i in range(2):
                        h = 2 * m + i
                        if s < NSTEPS_A:
                            nc.tensor.matmul(
                                out=ots[i], lhsT=V[s][:, h, :],
                                rhs=pts[i][:, 0:512],
                                start=(s == 0), stop=False,
                                skip_group_check=True)
                        else:
                            nc.tensor.matmul(
                                out=ots[i][:, QCH:2 * QCH],
                                lhsT=V[s][:, h, :], rhs=pts[i][:, 0:QCH],
                                start=False, stop=(s == NSTEPS_B - 1),
                                skip_group_check=True)
                if pending is not None:
                    normalize_pair(*pending)
                pending = (m, ots)
            normalize_pair(*pending)

            # ---- phase D: projection (two f-half groups per output) -------
            for qm in range(QT_N // 128):
                for (off, w) in ((0, 512), (512, 256)):
                    acc1 = psp.tile([128, 512], F32, tag="acc", name="acc1")
                    for f in range(3):
                        nc.tensor.matmul(
                            out=acc1[:, 0:w],
                            lhsT=YT[f][:, 128 * qm:128 * (qm + 1)],
                            rhs=wp_s[:, f, off:off + w],
                            start=(f == 0), stop=(f == 2))
                    acc2 = psp.tile([128, 512], F32, tag="acc", name="acc2")
                    for f in range(3, CT):
                        nc.tensor.matmul(
                            out=acc2[:, 0:w],
                            lhsT=YT[f][:, 128 * qm:128 * (qm + 1)],
                            rhs=wp_s[:, f, off:off + w],
                            start=(f == 3), stop=(f == CT - 1))
                    zt1 = zsp.tile([128, 512], F32, tag="zt1", name="zt1")
                    nc.vector.tensor_copy(out=zt1[:, 0:w], in_=acc1[:, 0:w])
                    zt = zsp.tile([128, 512], F32, tag="zt", name="zt")
                    nc.vector.tensor_add(zt[:, 0:w], acc2[:, 0:w],
                                         zt1[:, 0:w])
                    if with_bias:
                        nc.vector.tensor_add(zt[:, 0:w], zt[:, 0:w],
                                             bpb[:, off:off + w])
                    nc.sync.dma_start(
                        out=z_out[128 * qm:128 * (qm + 1), off:off + w],
                        in_=zt[:, 0:w])

    nc.finalize()
    return nc


def _prep_inputs(x, W_qkv, b_qkv, W_proj, b_proj):
    bf16 = ml_dtypes.bfloat16
    x = np.ascontiguousarray(np.asarray(x, dtype=np.float32))
    W_qkv = np.asarray(W_qkv, dtype=np.float32)
    b_qkv = np.asarray(b_qkv, dtype=np.float32)
    W_proj = np.asarray(W_proj, dtype=np.float32)
    b_proj = np.asarray(b_proj, dtype=np.float32)

    wqkv_b = np.ascontiguousarray(W_qkv.astype(bf16))
    wproj_b = np.ascontiguousarray(W_proj.astype(bf16))
    # b_qkv [2304] -> [128, 18] with [p, m] = b[128m + p]
    bqkv_t = np.ascontiguousarray(b_qkv.reshape(18, 128).T)
    bproj_t = np.ascontiguousarray(b_proj.reshape(1, C))
    xb = [np.ascontiguousarray(x[b].astype(bf16)) for b in range(B)]

    in_maps = []
    p = np.arange(128)
    for d in range(NCORES):
        b, j = d // 4, d % 4
        rows_a = np.arange(QCH * j, QCH * (j + 1))
        rows_b = np.arange(QCH * (7 - j), QCH * (8 - j))
        qrows = np.concatenate([rows_a, rows_b])
        xq = np.ascontiguousarray(xb[b][qrows])
        # masks [128, 16, 512]: steps 0-7 cols 0:256 = chunk A ktile s,
        # cols 256:512 = chunk B ktile s (all ones); steps 8-15 cols
        # 0:256 = chunk B ktile s (tail), cols 256:512 unused (ones).
        m = np.ones((128, NSTEPS_B, 2 * QCH), dtype=np.float32)
        for s in range(NSTEPS_B):
            kabs = (128 * s + p)[:, None]
            if s < NSTEPS_A:
                m[:, s, 0:QCH] = rows_a[None, :] >= kabs
                m[:, s, QCH:] = rows_b[None, :] >= kabs
            else:
                m[:, s, 0:QCH] = rows_b[None, :] >= kabs
        in_maps.append({
            "xbf": xb[b],
            "xqbf": xq,
            "wqkv": wqkv_b,
            "wproj": wproj_b,
            "bqkv": bqkv_t,
            "bproj": bproj_t,
            "masks": np.ascontiguousarray(m.astype(bf16)),
            "ident_in": np.eye(128, dtype=np.float32).astype(bf16),
        })
    return in_maps


def kernel(x, W_qkv, b_qkv, W_proj, b_proj):
    import os
    from concourse.bass_utils import run_bass_kernel_spmd

    in_maps = _prep_inputs(x, W_qkv, b_qkv, W_proj, b_proj)
    with_bias = bool(np.any(np.asarray(b_qkv)) or np.any(np.asarray(b_proj)))
    key = f"nc{with_bias}"
    if key not in _CACHE:
        _CACHE[key] = _build_program(with_bias)
    nc = _CACHE[key]
    res = run_bass_kernel_spmd(nc, in_maps, list(range(NCORES)),
                               trace=os.environ.get("KTRACE", "") == "1")
    _CACHE["last_result"] = res

    out = np.empty((B, T, C), dtype=np.float32)
    for d in range(NCORES):
        b, j = d // 4, d % 4
        z = np.asarray(res.results[d]["z"])
        out[b, QCH * j:QCH * (j + 1)] = z[0:QCH]
        out[b, QCH * (7 - j):QCH * (8 - j)] = z[QCH:2 * QCH]
    return out



# revision 9
# speedup vs baseline: 1.4989x; 1.4989x over previous
"""Causal self-attention (B=2, T=2048, C=768, H=12) on 8 NeuronCores.

Sharding per the hint: data-parallel on B (cores 0-3 batch 0, 4-7 batch 1)
x tensor-parallel over heads (core d%4 owns heads 3(d%4)..3(d%4)+2). Each
core computes q/k/v for ONLY its 3 heads, its heads' full causal TxT
attention, and a PARTIAL output projection (contraction over its 192 Y
columns). The 4 partials per batch are summed on the host (pure gather +
add) - zero device collectives.

Per-core pipeline (bf16 matmul operands, fp32 PSUM accumulation):
  x^T arrives pre-transposed from host (no on-device transposes at all).
  K^T/Q^T generated in 3 stationary-weight pair-matmuls ([128,2048] each:
  (k0|k1), (k2|q0), (q1|q2)), V in t-major [128, 3, 65] with a folded
  ones-row for the softmax denominator. Attention runs qtile-major
  (256-query tiles) with EXACT causal extents: qtile i processes step
  pairs sp=0..i, each [128, 512] = two 128-key tiles; only the last
  (diagonal) pair needs a mask multiply. exp on ACT (scale=1/8, no
  max-sub), P^T @ [V|1] accumulates [65, 256] per head into a shared
  [65, 768] PSUM tile, reciprocal + ones-matmul broadcast normalizes into
  Y^T, then the projection partial for the qtile's two 128-row t-tiles is
  copied to bf16 and DMAd out. K/Q/V generation is interleaved with the
  qtile loop so ACT/DVE spin up ~6us into the kernel.
"""

import numpy as np
import ml_dtypes

B, T, C, H, D = 2, 2048, 768, 12, 64
NCORES = 8
HPC = 3            # heads per core
QTW = 256          # query tile width
NQT = T // QTW     # 8 query tiles
CT = C // 128      # 6 contraction tiles

_CACHE = {}

# head -> (pair tile index, partition offset) for K^T and Q^T slices.
# pair tiles: 0 = (k0|k1), 1 = (q0|q1), 2 = (k2|q2); tile 3 = q2 copy [64,T]
_KSL = [(0, 0), (0, 64), (2, 0)]
_QSL = [(1, 0), (1, 64), (3, 0)]


def _build_program(with_bias=True):
    import concourse.bass as bass
    import concourse.bacc as bacc
    import concourse.mybir as mybir
    import concourse.tile as tile

    F32 = mybir.dt.float32
    BF16 = mybir.dt.bfloat16
    AF = mybir.ActivationFunctionType

    nc = bacc.Bacc()
    xT_in = nc.declare_dram_parameter("xT", [CT, 128, T], BF16, isOutput=False)
    wkq_in = nc.declare_dram_parameter("wkq", [CT, 128, 384], BF16,
                                       isOutput=False)
    wv_in = nc.declare_dram_parameter("wv", [CT, 128, 192], BF16,
                                      isOutput=False)
    wp_in = nc.declare_dram_parameter("wp", [2, 128, C], BF16, isOutput=False)
    masks_in = nc.declare_dram_parameter("masks", [128, 512], BF16,
                                         isOutput=False)
    if with_bias:
        bkq_in = nc.declare_dram_parameter("bkq", [128, 3], F32,
                                           isOutput=False)
    z_out = nc.declare_dram_parameter("z", [T, C], BF16, isOutput=True)

    scale = 1.0 / float(np.sqrt(D))

    with tile.TileContext(nc) as tc:
        with tc.tile_pool(name="const", bufs=1) as constp, \
             tc.tile_pool(name="data", bufs=1) as datap, \
             tc.tile_pool(name="pt", bufs=3) as ptp, \
             tc.tile_pool(name="small", bufs=3) as smallp, \
             tc.tile_pool(name="zs", bufs=2) as zsp, \
             tc.tile_pool(name="ps", bufs=3, space="PSUM") as psp, \
             tc.tile_pool(name="pot", bufs=1, space="PSUM") as potp, \
             tc.tile_pool(name="prb", bufs=1, space="PSUM") as prbp, \
             tc.tile_pool(name="ppj", bufs=1, space="PSUM") as ppjp:

            # ---- constants ------------------------------------------------
            wkq_s = constp.tile([128, CT, 384], BF16, tag="wkq")
            wv_s = constp.tile([128, CT, 192], BF16, tag="wv")
            wp_s = constp.tile([128, 2, C], BF16, tag="wp")
            masks_s = constp.tile([128, 512], BF16, tag="masks")
            ones1 = constp.tile([1, 64], BF16, tag="ones1")
            nc.vector.memset(ones1, 1.0)
            if with_bias:
                bkq_s = constp.tile([128, 3], F32, tag="bkq")

            # ---- persistent data ------------------------------------------
            xT = [datap.tile([128, T], BF16, tag=f"xT{c}", name=f"xT{c}")
                  for c in range(CT)]
            KQ = [datap.tile([128, T], BF16, tag=f"KQ{j}", name=f"KQ{j}")
                  for j in range(3)]
            KQ.append(datap.tile([64, T], BF16, tag="KQ3", name="KQ3"))
            V = [datap.tile([128, HPC, D + 1], BF16, tag=f"V{t}",
                            name=f"V{t}") for t in range(T // 128)]
            YT0 = datap.tile([128, T], BF16, tag="YT0", name="YT0")
            YT1 = datap.tile([64, T], BF16, tag="YT1", name="YT1")

            # ---- input DMAs (priority order) ------------------------------
            # scalar queue: weights + masks; sync queue: xT halves
            nc.scalar.dma_start(
                out=wkq_s,
                in_=bass.AP(tensor=wkq_in[:, :, :].tensor,
                            offset=wkq_in[:, :, :].offset,
                            ap=[[384, 128], [128 * 384, CT], [1, 384]]))
            for half in range(2):
                lo = (T // 2) * half
                for c in range(CT):
                    nc.sync.dma_start(
                        out=xT[c][:, lo:lo + T // 2],
                        in_=xT_in[c, :, lo:lo + T // 2])
            nc.scalar.dma_start(
                out=wv_s,
                in_=bass.AP(tensor=wv_in[:, :, :].tensor,
                            offset=wv_in[:, :, :].offset,
                            ap=[[192, 128], [128 * 192, CT], [1, 192]]))
            nc.scalar.dma_start(out=masks_s, in_=masks_in[:, :])
            nc.scalar.dma_start(
                out=wp_s,
                in_=bass.AP(tensor=wp_in[:, :, :].tensor,
                            offset=wp_in[:, :, :].offset,
                            ap=[[C, 128], [128 * C, 2], [1, C]]))
            if with_bias:
                nc.gpsimd.dma_start(out=bkq_s, in_=bkq_in[:, :])

            def gen_kq(tcnk):
                lo = 512 * tcnk
                for j in range(3):
                    acc = psp.tile([128, 512], F32, tag="acc", name="acc")
                    for c in range(CT):
                        nc.tensor.matmul(
                            out=acc,
                            lhsT=wkq_s[:, c, 128 * j:128 * (j + 1)],
                            rhs=xT[c][:, lo:lo + 512],
                            start=(c == 0), stop=(c == CT - 1))
                    if with_bias:
                        nc.vector.tensor_scalar_add(
                            KQ[j][:, lo:lo + 512], in0=acc,
                            scalar1=bkq_s[:, j:j + 1])
                    else:
                        nc.vector.tensor_copy(out=KQ[j][:, lo:lo + 512],
                                              in_=acc)
                # peel q2 (partitions 64:128 of the (k2|q2) pair) into its
                # own base-0 tile so S(h2) operands share a base partition
                nc.vector.tensor_copy(out=KQ[3][0:64, lo:lo + 512],
                                      in_=KQ[2][64:128, lo:lo + 512])

            def gen_v(t):
                acc = psp.tile([128, 512], F32, tag="acc", name="accv")
                for c in range(CT):
                    nc.tensor.matmul(
                        out=acc[:, 0:192],
                        lhsT=xT[c][:, 128 * t:128 * (t + 1)],
                        rhs=wv_s[:, c, :],
                        start=(c == 0), stop=(c == CT - 1))
                nc.vector.tensor_copy(out=V[t][:, :, 0:D], in_=acc[:, 0:192])
                nc.vector.memset(V[t][:, :, D:D + 1], 1.0)

            # ---- main loop: interleave generation with attention ----------
            for i in range(NQT):
                if i % 2 == 0:
                    gen_kq(i // 2)
                    for t in range(4 * (i // 2), 4 * (i // 2) + 4):
                        gen_v(t)

                qsl = slice(QTW * i, QTW * (i + 1))
                ot = potp.tile([65, 3 * QTW], F32, tag="ot", name="ot")
                pts = {}
                for sp in range(i + 1):
                    for h in range(HPC):
                        jk, pk = _KSL[h]
                        jq, pq = _QSL[h]
                        sps = psp.tile([128, 512], F32, tag="acc",
                                       name="sps")
                        for half in range(2):
                            klo = 256 * sp + 128 * half
                            nc.tensor.matmul(
                                out=sps[:, 256 * half:256 * (half + 1)],
                                lhsT=KQ[jk][pk:pk + 64, klo:klo + 128],
                                rhs=KQ[jq][pq:pq + 64, qsl],
                                start=True, stop=True)
                        pt = ptp.tile([128, 512], BF16, tag="pt", name="pt")
                        nc.scalar.activation(out=pt, in_=sps, func=AF.Exp,
                                             scale=scale)
                        if sp == i:
                            nc.vector.tensor_mul(pt, pt, masks_s)
                        pts[h] = pt
                    for h in range(HPC):
                        hsl = slice(QTW * h, QTW * (h + 1))
                        for half in range(2):
                            nc.tensor.matmul(
                                out=ot[:, hsl],
                                lhsT=V[2 * sp + half][:, h, :],
                                rhs=pts[h][:, 256 * half:256 * (half + 1)],
                                start=(sp == 0 and half == 0),
                                stop=(sp == i and half == 1),
                                skip_group_check=True)
                for h in range(HPC):
                    hsl = slice(QTW * h, QTW * (h + 1))
                    rec = smallp.tile([1, QTW], F32, tag="rec", name="rec")
                    nc.vector.reciprocal(out=rec, in_=ot[64:65, hsl])
                    recbf = smallp.tile([1, QTW], BF16, tag="recbf",
                                        name="recbf")
                    nc.vector.tensor_copy(out=recbf, in_=rec)
                    recb = prbp.tile([64, QTW], F32, tag="recb", name="recb")
                    nc.tensor.matmul(out=recb, lhsT=ones1, rhs=recbf,
                                     start=True, stop=True)
                    recb_sb = smallp.tile([64, QTW], F32, tag="recb_sb",
                                          name="recb_sb")
                    nc.vector.tensor_copy(out=recb_sb, in_=recb)
                    ysl = (YT0[0:64, qsl] if h == 0 else
                           YT0[64:128, qsl] if h == 1 else YT1[0:64, qsl])
                    nc.vector.tensor_mul(ysl, ot[0:64, hsl], recb_sb)

                for tt in (2 * i, 2 * i + 1):
                    tsl = slice(128 * tt, 128 * (tt + 1))
                    pacc = ppjp.tile([128, C], F32, tag="pacc", name="pacc")
                    nc.tensor.matmul(out=pacc, lhsT=YT0[:, tsl],
                                     rhs=wp_s[:, 0, :], start=True,
                                     stop=False)
                    nc.tensor.matmul(out=pacc, lhsT=YT1[:, tsl],
                                     rhs=wp_s[0:64, 1, :], start=False,
                                     stop=True)
                    zt = zsp.tile([128, C], BF16, tag="zt", name="zt")
                    nc.vector.tensor_copy(out=zt, in_=pacc)
                    nc.gpsimd.dma_start(out=z_out[tsl, :], in_=zt)

    nc.finalize()
    return nc


def _prep_inputs(x, W_qkv, b_qkv, W_proj, b_proj):
    bf16 = ml_dtypes.bfloat16
    x = np.asarray(x, dtype=np.float32)
    W_qkv = np.asarray(W_qkv, dtype=np.float32)
    b_qkv = np.asarray(b_qkv, dtype=np.float32)

    # masks [128, 512]: col 256*d + q valid iff q >= 128*d + p
    p = np.arange(128)[:, None]
    q = np.arange(QTW)[None, :]
    m = np.ones((128, 512), dtype=np.float32)
    m[:, 0:QTW] = q >= p
    m[:, QTW:512] = q >= 128 + p
    m_bf = np.ascontiguousarray(m.astype(bf16))

    xTb = [np.ascontiguousarray(
        x[b].T.astype(bf16).reshape(CT, 128, T)) for b in range(B)]

    in_maps = []
    for d in range(NCORES):
        b, g = d // 4, d % 4
        qcols = W_qkv[:, 192 * g:192 * (g + 1)]
        kcols = W_qkv[:, C + 192 * g:C + 192 * (g + 1)]
        vcols = W_qkv[:, 2 * C + 192 * g:2 * C + 192 * (g + 1)]
        wkq = np.concatenate(
            [kcols[:, 0:128], qcols[:, 0:128], kcols[:, 128:192],
             qcols[:, 128:192]], axis=1)         # [768, 384]
        wp = np.zeros((256, C), dtype=np.float32)
        wp[0:192] = W_proj[192 * g:192 * (g + 1), :]
        qb = b_qkv[192 * g:192 * (g + 1)]
        kb = b_qkv[C + 192 * g:C + 192 * (g + 1)]
        bkq = np.stack([kb[0:128], qb[0:128],
                        np.concatenate([kb[128:192], qb[128:192]])],
                       axis=1)                   # [128, 3]
        in_maps.append({
            "xT": xTb[b],
            "wkq": np.ascontiguousarray(wkq.astype(bf16).reshape(CT, 128, 384)),
            "wv": np.ascontiguousarray(vcols.astype(bf16).reshape(CT, 128, 192)),
            "wp": np.ascontiguousarray(wp.astype(bf16).reshape(2, 128, C)),
            "masks": m_bf,
            "bkq": np.ascontiguousarray(bkq.astype(np.float32)),
        })
    return in_maps


def kernel(x, W_qkv, b_qkv, W_proj, b_proj):
    import os
    from concourse.bass_utils import run_bass_kernel_spmd

    b_qkv = np.asarray(b_qkv, dtype=np.float32)
    b_proj = np.asarray(b_proj, dtype=np.float32)
    W_qkv = np.asarray(W_qkv, dtype=np.float32)
    W_proj = np.asarray(W_proj, dtype=np.float32)
    in_maps = _prep_inputs(x, W_qkv, b_qkv, W_proj, b_proj)
    with_bias = bool(np.any(b_qkv[0:2 * C]))
    if not with_bias:
        for im in in_maps:
            del im["bkq"]
    key = f"nc{with_bias}"
    if key not in _CACHE:
        _CACHE[key] = _build_program(with_bias)
    nc = _CACHE[key]
    res = run_bass_kernel_spmd(nc, in_maps, list(range(NCORES)),
                               trace=os.environ.get("KTRACE", "") == "1")
    _CACHE["last_result"] = res

    # host-side unshard: sum the 4 head-group partials per batch.
    out = np.empty((B, T, C), dtype=np.float32)
    for b in range(B):
        acc = np.zeros((T, C), dtype=np.float32)
        for g in range(4):
            acc += np.asarray(res.results[4 * b + g]["z"]).astype(np.float32)
        # v-bias and proj-bias fold in linearly on the host:
        # out = P(V + bv) Wp + bp = (PV) Wp + bv Wp + bp
        bv = b_qkv[2 * C:3 * C]
        out[b] = acc + (bv @ W_proj + b_proj)[None, :]
    return out


# revision 12
# speedup vs baseline: 1.6673x; 1.1123x over previous
"""Causal self-attention (B=2, T=2048, C=768, H=12) on 8 NeuronCores.

Sharding per the hint: data-parallel on B (cores 0-3 batch 0, 4-7 batch 1)
x tensor-parallel over heads (core d%4 owns heads 3(d%4)..3(d%4)+2). Each
core computes q/k/v for ONLY its 3 heads, its heads' full causal TxT
attention, and a PARTIAL output projection (contraction over its 192 Y
columns). The 4 partials per batch are summed on the host (pure gather +
add) - zero device collectives.

Per-core pipeline (bf16 matmul operands, fp32 PSUM accumulation):
  x^T arrives pre-transposed from host (no on-device transposes at all).
  K^T/Q^T generated in 3 stationary-weight pair-matmuls ([128,2048] each:
  (k0|k1), (k2|q0), (q1|q2)), V in t-major [128, 3, 65] with a folded
  ones-row for the softmax denominator. Attention runs qtile-major
  (256-query tiles) with EXACT causal extents: qtile i processes step
  pairs sp=0..i, each [128, 512] = two 128-key tiles; only the last
  (diagonal) pair needs a mask multiply. exp on ACT (scale=1/8, no
  max-sub), P^T @ [V|1] accumulates [65, 256] per head into a shared
  [65, 768] PSUM tile, reciprocal + ones-matmul broadcast normalizes into
  Y^T, then the projection partial for the qtile's two 128-row t-tiles is
  copied to bf16 and DMAd out. K/Q/V generation is interleaved with the
  qtile loop so ACT/DVE spin up ~6us into the kernel.
"""

import numpy as np
import ml_dtypes

B, T, C, H, D = 2, 2048, 768, 12, 64
NCORES = 8
HPC = 3            # heads per core
QTW = 256          # query tile width
NQT = T // QTW     # 8 query tiles
CT = C // 128      # 6 contraction tiles

_CACHE = {}

# head -> (pair tile index, partition offset) for K^T and Q^T slices.
# pair tiles: 0 = (k0|k1), 1 = (q0|q1), 2 = (k2|q2); tile 3 = q2 copy [64,T]
_KSL = [(0, 0), (0, 64), (2, 0)]
_QSL = [(1, 0), (1, 64), (3, 0)]


def _build_program(with_bias=True):
    import concourse.bass as bass
    import concourse.bacc as bacc
    import concourse.mybir as mybir
    import concourse.tile as tile

    F32 = mybir.dt.float32
    BF16 = mybir.dt.bfloat16
    AF = mybir.ActivationFunctionType

    nc = bacc.Bacc()
    xT_in = nc.declare_dram_parameter("xT", [CT, 128, T], BF16, isOutput=False)
    wkq_in = nc.declare_dram_parameter("wkq", [CT, 128, 384], BF16,
                                       isOutput=False)
    wv_in = nc.declare_dram_parameter("wv", [CT, 128, 192], BF16,
                                      isOutput=False)
    wp_in = nc.declare_dram_parameter("wp", [2, 128, C], BF16, isOutput=False)
    masks_in = nc.declare_dram_parameter("masks", [128, 512], BF16,
                                         isOutput=False)
    if with_bias:
        bkq_in = nc.declare_dram_parameter("bkq", [128, 3], F32,
                                           isOutput=False)
    z_out = nc.declare_dram_parameter("z", [T, C], BF16, isOutput=True)

    scale = 1.0 / float(np.sqrt(D))

    with tile.TileContext(nc) as tc:
        with tc.tile_pool(name="const", bufs=1) as constp, \
             tc.tile_pool(name="data", bufs=1) as datap, \
             tc.tile_pool(name="pt", bufs=3) as ptp, \
             tc.tile_pool(name="small", bufs=3) as smallp, \
             tc.tile_pool(name="zs", bufs=2) as zsp, \
             tc.tile_pool(name="ps", bufs=2, space="PSUM") as psp, \
             tc.tile_pool(name="pot", bufs=2, space="PSUM") as potp, \
             tc.tile_pool(name="ppj", bufs=1, space="PSUM") as ppjp:

            # ---- constants ------------------------------------------------
            wkq_s = constp.tile([128, CT, 384], BF16, tag="wkq")
            wv_s = constp.tile([128, CT, 192], BF16, tag="wv")
            wp_s = constp.tile([128, 2, C], BF16, tag="wp")
            masks_s = constp.tile([128, 512], BF16, tag="masks")
            ones1 = constp.tile([1, 64], BF16, tag="ones1")
            nc.vector.memset(ones1, 1.0)
            if with_bias:
                bkq_s = constp.tile([128, 3], F32, tag="bkq")

            # ---- persistent data ------------------------------------------
            xT = [datap.tile([128, T], BF16, tag=f"xT{c}", name=f"xT{c}")
                  for c in range(CT)]
            KQ = [datap.tile([128, T], BF16, tag=f"KQ{j}", name=f"KQ{j}")
                  for j in range(3)]
            KQ.append(datap.tile([64, T], BF16, tag="KQ3", name="KQ3"))
            V = [datap.tile([128, HPC, D + 1], BF16, tag=f"V{t}",
                            name=f"V{t}") for t in range(T // 128)]
            YT0 = datap.tile([128, T], BF16, tag="YT0", name="YT0")
            YT1 = datap.tile([64, T], BF16, tag="YT1", name="YT1")

            # ---- input DMAs (priority order) ------------------------------
            # scalar queue: weights + masks; sync queue: xT halves
            nc.scalar.dma_start(
                out=wkq_s,
                in_=bass.AP(tensor=wkq_in[:, :, :].tensor,
                            offset=wkq_in[:, :, :].offset,
                            ap=[[384, 128], [128 * 384, CT], [1, 384]]))
            for half in range(2):
                lo = (T // 2) * half
                for c in range(CT):
                    nc.sync.dma_start(
                        out=xT[c][:, lo:lo + T // 2],
                        in_=xT_in[c, :, lo:lo + T // 2])
            nc.scalar.dma_start(
                out=wv_s,
                in_=bass.AP(tensor=wv_in[:, :, :].tensor,
                            offset=wv_in[:, :, :].offset,
                            ap=[[192, 128], [128 * 192, CT], [1, 192]]))
            nc.scalar.dma_start(out=masks_s, in_=masks_in[:, :])
            nc.scalar.dma_start(
                out=wp_s,
                in_=bass.AP(tensor=wp_in[:, :, :].tensor,
                            offset=wp_in[:, :, :].offset,
                            ap=[[C, 128], [128 * C, 2], [1, C]]))
            if with_bias:
                nc.gpsimd.dma_start(out=bkq_s, in_=bkq_in[:, :])

            def gen_kq(tcnk):
                lo = 512 * tcnk
                for j in range(3):
                    acc = psp.tile([128, 512], F32, tag="acc", name="acc")
                    for c in range(CT):
                        nc.tensor.matmul(
                            out=acc,
                            lhsT=wkq_s[:, c, 128 * j:128 * (j + 1)],
                            rhs=xT[c][:, lo:lo + 512],
                            start=(c == 0), stop=(c == CT - 1))
                    if with_bias:
                        nc.vector.tensor_scalar_add(
                            KQ[j][:, lo:lo + 512], in0=acc,
                            scalar1=bkq_s[:, j:j + 1])
                    else:
                        nc.vector.tensor_copy(out=KQ[j][:, lo:lo + 512],
                                              in_=acc)
                # peel q2 (partitions 64:128 of the (k2|q2) pair) into its
                # own base-0 tile so S(h2) operands share a base partition
                nc.vector.tensor_copy(out=KQ[3][0:64, lo:lo + 512],
                                      in_=KQ[2][64:128, lo:lo + 512])

            def gen_v(t):
                acc = psp.tile([128, 512], F32, tag="acc", name="accv")
                for c in range(CT):
                    nc.tensor.matmul(
                        out=acc[:, 0:192],
                        lhsT=xT[c][:, 128 * t:128 * (t + 1)],
                        rhs=wv_s[:, c, :],
                        start=(c == 0), stop=(c == CT - 1))
                nc.vector.tensor_copy(out=V[t][:, :, 0:D], in_=acc[:, 0:192])
                nc.vector.memset(V[t][:, :, D:D + 1], 1.0)

            # ---- main loop: interleave generation with attention ----------
            for i in range(NQT):
                if i % 2 == 0:
                    gen_kq(i // 2)
                    for t in range(4 * (i // 2), 4 * (i // 2) + 4):
                        gen_v(t)

                qsl = slice(QTW * i, QTW * (i + 1))
                ot = potp.tile([65, 3 * QTW], F32, tag="ot", name="ot")
                pts = {}
                for sp in range(i + 1):
                    for h in range(HPC):
                        jk, pk = _KSL[h]
                        jq, pq = _QSL[h]
                        sps = psp.tile([128, 512], F32, tag="acc",
                                       name="sps")
                        for half in range(2):
                            klo = 256 * sp + 128 * half
                            nc.tensor.matmul(
                                out=sps[:, 256 * half:256 * (half + 1)],
                                lhsT=KQ[jk][pk:pk + 64, klo:klo + 128],
                                rhs=KQ[jq][pq:pq + 64, qsl],
                                start=True, stop=True)
                        pt = ptp.tile([128, 512], BF16, tag="pt", name="pt")
                        nc.scalar.activation(out=pt, in_=sps, func=AF.Exp,
                                             scale=scale)
                        if sp == i:
                            nc.vector.tensor_mul(pt, pt, masks_s)
                        pts[h] = pt
                    for h in range(HPC):
                        hsl = slice(QTW * h, QTW * (h + 1))
                        for half in range(2):
                            nc.tensor.matmul(
                                out=ot[:, hsl],
                                lhsT=V[2 * sp + half][:, h, :],
                                rhs=pts[h][:, 256 * half:256 * (half + 1)],
                                start=(sp == 0 and half == 0),
                                stop=(sp == i and half == 1),
                                skip_group_check=True)
                for h in range(HPC):
                    hsl = slice(QTW * h, QTW * (h + 1))
                    rec = smallp.tile([1, QTW], F32, tag="rec", name="rec")
                    nc.vector.reciprocal(out=rec, in_=ot[64:65, hsl])
                    recbf = smallp.tile([1, QTW], BF16, tag="recbf",
                                        name="recbf")
                    nc.vector.tensor_copy(out=recbf, in_=rec)
                    recb = ppjp.tile([64, QTW], F32, tag="pacc", name="recb")
                    nc.tensor.matmul(out=recb, lhsT=ones1, rhs=recbf,
                                     start=True, stop=True)
                    recb_sb = smallp.tile([64, QTW], F32, tag="recb_sb",
                                          name="recb_sb")
                    nc.vector.tensor_copy(out=recb_sb, in_=recb)
                    ysl = (YT0[0:64, qsl] if h == 0 else
                           YT0[64:128, qsl] if h == 1 else YT1[0:64, qsl])
                    nc.vector.tensor_mul(ysl, ot[0:64, hsl], recb_sb)

                for tt in (2 * i, 2 * i + 1):
                    tsl = slice(128 * tt, 128 * (tt + 1))
                    pacc = ppjp.tile([128, C], F32, tag="pacc", name="pacc")
                    nc.tensor.matmul(out=pacc, lhsT=YT0[:, tsl],
                                     rhs=wp_s[:, 0, :], start=True,
                                     stop=False)
                    nc.tensor.matmul(out=pacc, lhsT=YT1[:, tsl],
                                     rhs=wp_s[0:64, 1, :], start=False,
                                     stop=True)
                    zt = zsp.tile([128, C], BF16, tag="zt", name="zt")
                    nc.vector.tensor_copy(out=zt, in_=pacc)
                    nc.gpsimd.dma_start(out=z_out[tsl, :], in_=zt)

    nc.finalize()
    return nc


def _prep_inputs(x, W_qkv, b_qkv, W_proj, b_proj):
    bf16 = ml_dtypes.bfloat16
    x = np.asarray(x, dtype=np.float32)
    W_qkv = np.asarray(W_qkv, dtype=np.float32)
    b_qkv = np.asarray(b_qkv, dtype=np.float32)

    # masks [128, 512]: col 256*d + q valid iff q >= 128*d + p
    p = np.arange(128)[:, None]
    q = np.arange(QTW)[None, :]
    m = np.ones((128, 512), dtype=np.float32)
    m[:, 0:QTW] = q >= p
    m[:, QTW:512] = q >= 128 + p
    m_bf = np.ascontiguousarray(m.astype(bf16))

    xTb = [np.ascontiguousarray(
        x[b].T.astype(bf16).reshape(CT, 128, T)) for b in range(B)]

    in_maps = []
    for d in range(NCORES):
        b, g = d // 4, d % 4
        qcols = W_qkv[:, 192 * g:192 * (g + 1)]
        kcols = W_qkv[:, C + 192 * g:C + 192 * (g + 1)]
        vcols = W_qkv[:, 2 * C + 192 * g:2 * C + 192 * (g + 1)]
        wkq = np.concatenate(
            [kcols[:, 0:128], qcols[:, 0:128], kcols[:, 128:192],
             qcols[:, 128:192]], axis=1)         # [768, 384]
        wp = np.zeros((256, C), dtype=np.float32)
        wp[0:192] = W_proj[192 * g:192 * (g + 1), :]
        qb = b_qkv[192 * g:192 * (g + 1)]
        kb = b_qkv[C + 192 * g:C + 192 * (g + 1)]
        bkq = np.stack([kb[0:128], qb[0:128],
                        np.concatenate([kb[128:192], qb[128:192]])],
                       axis=1)                   # [128, 3]
        in_maps.append({
            "xT": xTb[b],
            "wkq": np.ascontiguousarray(wkq.astype(bf16).reshape(CT, 128, 384)),
            "wv": np.ascontiguousarray(vcols.astype(bf16).reshape(CT, 128, 192)),
            "wp": np.ascontiguousarray(wp.astype(bf16).reshape(2, 128, C)),
            "masks": m_bf,
            "bkq": np.ascontiguousarray(bkq.astype(np.float32)),
        })
    return in_maps


def kernel(x, W_qkv, b_qkv, W_proj, b_proj):
    import os
    from concourse.bass_utils import run_bass_kernel_spmd

    b_qkv = np.asarray(b_qkv, dtype=np.float32)
    b_proj = np.asarray(b_proj, dtype=np.float32)
    W_qkv = np.asarray(W_qkv, dtype=np.float32)
    W_proj = np.asarray(W_proj, dtype=np.float32)
    in_maps = _prep_inputs(x, W_qkv, b_qkv, W_proj, b_proj)
    with_bias = bool(np.any(b_qkv[0:2 * C]))
    if not with_bias:
        for im in in_maps:
            del im["bkq"]
    key = f"nc{with_bias}"
    if key not in _CACHE:
        _CACHE[key] = _build_program(with_bias)
    nc = _CACHE[key]
    res = run_bass_kernel_spmd(nc, in_maps, list(range(NCORES)),
                               trace=os.environ.get("KTRACE", "") == "1")
    _CACHE["last_result"] = res

    # host-side unshard: sum the 4 head-group partials per batch.
    out = np.empty((B, T, C), dtype=np.float32)
    for b in range(B):
        acc = np.zeros((T, C), dtype=np.float32)
        for g in range(4):
            acc += np.asarray(res.results[4 * b + g]["z"]).astype(np.float32)
        # v-bias and proj-bias fold in linearly on the host:
        # out = P(V + bv) Wp + bp = (PV) Wp + bv Wp + bp
        bv = b_qkv[2 * C:3 * C]
        out[b] = acc + (bv @ W_proj + b_proj)[None, :]
    return out
